# revision 1
# baseline (speedup 1.0000x reference)
"""GRU cell on 8 Trainium2 NeuronCores.

Reference computation (B=65536, D=256):
    z = sigmoid(x@Wz + h@Uz + bz)
    r = sigmoid(x@Wr + h@Ur + br)
    h_hat = tanh(x@Wh + (r*h)@Uh + bh)
    h_t = z*h + (1-z)*h_hat  ; returns (h_t, h_t)

Strategy: data-parallel over the batch dim (8 shards of 8192 rows).
The host pre-transposes each shard to [256, B_shard] so every on-chip
tensor lives in [hidden, batch] layout: the contraction dim of all six
GEMMs is then the SBUF partition dim with no on-chip transposes at all,
biases become per-partition ACT bias vectors, and the elementwise gate
math runs in the same layout the matmuls produce.  Matmul operands are
bitcast to float32r (full-rate PE mode for fp32 data).
"""

import os
import sys

for _p in ("/opt/trn_rl_repo", "/root/.axon_site/_ro/trn_rl_repo"):
    if os.path.isdir(_p) and _p not in sys.path:
        sys.path.append(_p)

import numpy as np

B = 65536
D = 256
N_CORES = 8
S = B // N_CORES  # batch rows per core
CH = 512  # batch columns per chunk (one PSUM bank of fp32)

_WNAMES = ("Wz", "Uz", "Wr", "Ur", "Wh", "Uh")
_BNAMES = ("bz", "br", "bh")


def build_nc(s=S, mm_dtype_name=None, ch=CH):
    """Build + compile the per-core Bass program for a shard of s rows."""
    import concourse.bass as bass
    import concourse.mybir as mybir
    import concourse.tile as tile
    from concourse import bacc

    f32 = mybir.dt.float32
    if mm_dtype_name is None:
        mm_dtype_name = os.environ.get("GRU_MM_DTYPE", "float32r")
    mm_dt = getattr(mybir.dt, mm_dtype_name)
    AF = mybir.ActivationFunctionType

    nc = bacc.Bacc("TRN2", target_bir_lowering=False)
    xT = nc.dram_tensor("xT", [D, s], f32, kind="ExternalInput")
    hT = nc.dram_tensor("hT", [D, s], f32, kind="ExternalInput")
    w_d = {n: nc.dram_tensor(n, [D, D], f32, kind="ExternalInput") for n in _WNAMES}
    b_d = {n: nc.dram_tensor(n, [D], f32, kind="ExternalInput") for n in _BNAMES}
    outT = nc.dram_tensor("outT", [D, s], f32, kind="ExternalOutput")

    nch = s // ch
    cast = mm_dt != f32
    # float32r is bit-identical to float32; allocate matmul operand tiles as
    # f32r and bitcast the fp32 views where engines need plain f32 semantics.
    f32r_mode = mm_dt == mybir.dt.float32r

    def md(ap):
        if ap.dtype == mm_dt:
            return ap
        return ap.bitcast(mm_dt) if cast else ap

    with tile.TileContext(nc) as tc:
        with (
            tc.tile_pool(name="const", bufs=1) as cpool,
            tc.tile_pool(name="inp", bufs=3) as ipool,
            tc.tile_pool(name="work", bufs=3) as wpool,
            tc.tile_pool(name="psum", bufs=1, space=bass.MemorySpace.PSUM) as ppool,
        ):
            # --- constants: weights [128, 256] x2 k-chunks each, biases [128, 2]
            w_sb = {}
            for n in _WNAMES:
                for k in range(2):
                    src = w_d[n][k * 128 : (k + 1) * 128, :]
                    if f32r_mode:
                        t = cpool.tile([128, D], mm_dt, tag=f"w_{n}_{k}")
                        nc.sync.dma_start(t[:], src.bitcast(mm_dt))
                    elif cast:
                        t0 = cpool.tile([128, D], f32, tag=f"wld_{n}_{k}")
                        nc.sync.dma_start(t0[:], src)
                        t = cpool.tile([128, D], mm_dt, tag=f"w_{n}_{k}")
                        nc.vector.tensor_copy(t[:], t0[:])
                    else:
                        t = cpool.tile([128, D], f32, tag=f"w_{n}_{k}")
                        nc.sync.dma_start(t[:], src)
                    w_sb[(n, k)] = t
            b_sb = {}
            for n in _BNAMES:
                t = cpool.tile([128, 2], f32, tag=f"b_{n}")
                nc.sync.dma_start(t[:], b_d[n].rearrange("(g p) -> p g", p=128))
                b_sb[n] = t

            def gate_psum(pool_tag, wn, un, rhs_w, rhs_u, g):
                """psum[{128},{ch}] = W[:,g].T @ rhs_w + U[:,g].T @ rhs_u."""
                p = ppool.tile([128, ch], f32, tag=pool_tag)
                gs = slice(g * 128, (g + 1) * 128)
                nc.tensor.matmul(p[:], md(w_sb[(wn, 0)][:, gs]), md(rhs_w[0][:]),
                                 start=True, stop=False)
                nc.tensor.matmul(p[:], md(w_sb[(wn, 1)][:, gs]), md(rhs_w[1][:]),
                                 start=False, stop=False)
                nc.tensor.matmul(p[:], md(w_sb[(un, 0)][:, gs]), md(rhs_u[0][:]),
                                 start=False, stop=False)
                nc.tensor.matmul(p[:], md(w_sb[(un, 1)][:, gs]), md(rhs_u[1][:]),
                                 start=False, stop=True)
                return p

            for c in range(nch):
                cols = slice(c * ch, (c + 1) * ch)
                # xt/ht: matmul-operand tiles; htf: f32 views of h for the
                # elementwise gate math.
                xt, ht, htf = [], [], []
                for k in range(2):
                    if f32r_mode:
                        tx = ipool.tile([128, ch], mm_dt, tag=f"x{k}")
                        nc.sync.dma_start(
                            tx[:], xT[k * 128 : (k + 1) * 128, cols].bitcast(mm_dt)
                        )
                        th = ipool.tile([128, ch], mm_dt, tag=f"h{k}")
                        nc.sync.dma_start(
                            th[:], hT[k * 128 : (k + 1) * 128, cols].bitcast(mm_dt)
                        )
                        xt.append(tx)
                        ht.append(th)
                        htf.append(th[:].bitcast(f32))
                    else:
                        tx = ipool.tile([128, ch], f32, tag=f"x{k}")
                        nc.sync.dma_start(tx[:], xT[k * 128 : (k + 1) * 128, cols])
                        th = ipool.tile([128, ch], f32, tag=f"h{k}")
                        nc.sync.dma_start(th[:], hT[k * 128 : (k + 1) * 128, cols])
                        htf.append(th[:])
                        if cast:
                            cx = ipool.tile([128, ch], mm_dt, tag=f"xc{k}")
                            nc.vector.tensor_copy(cx[:], tx[:])
                            chh = ipool.tile([128, ch], mm_dt, tag=f"hc{k}")
                            nc.vector.tensor_copy(chh[:], th[:])
                            xt.append(cx)
                            ht.append(chh)
                        else:
                            xt.append(tx)
                            ht.append(th)

                # reset gate -> r*h (needed before the candidate matmuls)
                rh = []
                for g in range(2):
                    pr = gate_psum(f"pr{g}", "Wr", "Ur", xt, ht, g)
                    rt = wpool.tile([128, ch], f32, tag=f"r{g}")
                    nc.scalar.activation(rt[:], pr[:], AF.Sigmoid,
                                         bias=b_sb["br"][:, g : g + 1])
                    t = wpool.tile([128, ch], mm_dt if cast else f32, tag=f"rh{g}")
                    nc.vector.tensor_mul(t[:], rt[:], htf[g])
                    rh.append(t)

                # update gate
                zt = []
                for g in range(2):
                    pz = gate_psum(f"pz{g}", "Wz", "Uz", xt, ht, g)
                    t = wpool.tile([128, ch], f32, tag=f"z{g}")
                    nc.scalar.activation(t[:], pz[:], AF.Sigmoid,
                                         bias=b_sb["bz"][:, g : g + 1])
                    zt.append(t)

                # candidate + combine + store
                for g in range(2):
                    ph = gate_psum(f"ph{g}", "Wh", "Uh", xt, rh, g)
                    hh = wpool.tile([128, ch], f32, tag=f"hh{g}")
                    nc.scalar.activation(hh[:], ph[:], AF.Tanh,
                                         bias=b_sb["bh"][:, g : g + 1])
                    d = wpool.tile([128, ch], f32, tag=f"d{g}")
                    nc.vector.tensor_sub(d[:], htf[g], hh[:])
                    m = wpool.tile([128, ch], f32, tag=f"m{g}")
                    nc.vector.tensor_mul(m[:], zt[g][:], d[:])
                    o = wpool.tile([128, ch], f32, tag=f"o{g}")
                    nc.vector.tensor_add(o[:], hh[:], m[:])
                    nc.sync.dma_start(outT[g * 128 : (g + 1) * 128, cols], o[:])

    nc.compile()
    return nc


_NC_CACHE = {}


def _get_nc():
    key = (S, os.environ.get("GRU_MM_DTYPE", "float32r"), CH)
    if key not in _NC_CACHE:
        _NC_CACHE[key] = build_nc(S, key[1], CH)
    return _NC_CACHE[key]


def _make_in_maps(inputs):
    f32 = np.float32
    x = np.asarray(inputs["x"], f32)
    h = np.asarray(inputs["h_t_1"], f32)
    consts = {n: np.ascontiguousarray(np.asarray(inputs[n], f32)) for n in _WNAMES}
    consts.update(
        {n: np.ascontiguousarray(np.asarray(inputs[n], f32)) for n in _BNAMES}
    )
    in_maps = []
    for c in range(N_CORES):
        sl = slice(c * S, (c + 1) * S)
        m = {
            "xT": np.ascontiguousarray(x[sl].T),
            "hT": np.ascontiguousarray(h[sl].T),
        }
        m.update(consts)
        in_maps.append(m)
    return in_maps


def run(inputs, trace=False):
    """Run on hardware; returns (h_t ndarray, BassKernelResults)."""
    from concourse.bass_utils import run_bass_kernel_spmd

    nc = _get_nc()
    in_maps = _make_in_maps(inputs)
    res = run_bass_kernel_spmd(nc, in_maps, list(range(N_CORES)), trace=trace)
    out = np.empty((B, D), np.float32)
    for c in range(N_CORES):
        out[c * S : (c + 1) * S] = res.results[c]["outT"].T
    return out, res


def kernel(**inputs):
    out, _ = run(inputs, trace=False)
    return (out, out)



# revision 3
# speedup vs baseline: 1.1730x; 1.1730x over previous
"""GRU cell on 8 Trainium2 NeuronCores.

Reference computation (B=65536, D=256):
    z = sigmoid(x@Wz + h@Uz + bz)
    r = sigmoid(x@Wr + h@Ur + br)
    h_hat = tanh(x@Wh + (r*h)@Uh + bh)
    h_t = z*h + (1-z)*h_hat  ; returns (h_t, h_t)

Strategy: data-parallel over the batch dim (8 shards of 8192 rows), with
everything on-chip in fp16:
  * host pre-transposes each shard to [256, B_shard] and casts to fp16 so
    the contraction dim of all six GEMMs is the SBUF partition dim,
  * fp16 halves HBM traffic (the fp32 baseline had all 16 DMA queues ~80%
    busy) and streams the PE at full rate with fast weight load,
  * weights are packed into one [256, 1536] fp16 matrix, loaded r-gate
    first so the first matmul can start ~2us in,
  * all input tiles are resident in SBUF (8.4MB of 24MB), so the DMA
    engines run free of WAR hazards from t=0,
  * gate math runs fp16 on DVE (2x mode, SBUF-only operands); activations
    read PSUM f32 and write fp16.
rel_l2 error of the all-fp16 pipeline vs the f32 reference is ~1.1e-3
(gate: 2e-2).
"""

import os
import sys

for _p in ("/opt/trn_rl_repo", "/root/.axon_site/_ro/trn_rl_repo"):
    if os.path.isdir(_p) and _p not in sys.path:
        sys.path.append(_p)

import numpy as np

B = 65536
D = 256
N_CORES = 8
S = B // N_CORES  # batch rows per core
CH = 512  # batch columns per PSUM bank / compute sub-chunk
OG = 2048  # batch columns per output staging tile

# Input-tile load plan: (col_start, width). The first two are narrow so the
# first sub-chunk's operands land quickly; the rest are wide for DMA
# efficiency (2KB+ per-partition lines).
PLAN = [(0, 512), (512, 512)] + [(1024 + 1024 * i, 1024) for i in range(7)]
# matrix order inside the packed weight tensor
_WORDER = ("Wr", "Ur", "Wz", "Uz", "Wh", "Uh")
_BORDER = ("br", "bz", "bh")


def _sub_to_load(j):
    """Map 512-wide sub-chunk j to (load_index, local col offset)."""
    c0 = j * CH
    for li, (start, width) in enumerate(PLAN):
        if start <= c0 < start + width:
            return li, c0 - start
    raise ValueError(j)


def build_nc(s=S, mm_dtype_name=None):
    """Build + compile the per-core Bass program for a shard of s rows."""
    import concourse.bass as bass
    import concourse.mybir as mybir
    import concourse.tile as tile
    from concourse import bacc

    f32 = mybir.dt.float32
    if mm_dtype_name is None:
        mm_dtype_name = os.environ.get("GRU_MM_DTYPE", "float16")
    f16 = getattr(mybir.dt, mm_dtype_name)
    AF = mybir.ActivationFunctionType

    nc = bacc.Bacc("TRN2", target_bir_lowering=False)
    xT = nc.dram_tensor("xT", [D, s], f16, kind="ExternalInput")
    hT = nc.dram_tensor("hT", [D, s], f16, kind="ExternalInput")
    wcat = nc.dram_tensor("wcat", [D, 6 * D], f16, kind="ExternalInput")
    bcat = nc.dram_tensor("bcat", [128, 6], f32, kind="ExternalInput")
    outT = nc.dram_tensor("outT", [D, s], f16, kind="ExternalOutput")

    nsub = s // CH

    with tile.TileContext(nc) as tc:
        with (
            tc.tile_pool(name="const", bufs=1) as cpool,
            tc.tile_pool(name="work", bufs=2) as wpool,
            tc.tile_pool(name="out", bufs=2) as opool,
            tc.tile_pool(name="psum", bufs=1, space=bass.MemorySpace.PSUM) as ppool,
        ):
            # --- weights: r-gate slice first (cols 0:512 = Wr|Ur), then the
            # rest (cols 512:1536 = Wz|Uz|Wh|Uh), per contraction k-half.
            wA, wB = {}, {}
            for k in range(2):
                wA[k] = cpool.tile([128, 2 * D], f16, tag=f"wA{k}", name=f"wA{k}")
                nc.sync.dma_start(wA[k][:], wcat[k * 128 : (k + 1) * 128, 0 : 2 * D])

            # --- first two input loads (narrow, fast start)
            xt, ht = {}, {}  # (k, load_idx) -> tile
            def load_inputs(li):
                start, width = PLAN[li]
                for k in range(2):
                    t = cpool.tile([128, width], f16, tag=f"x{k}_{li}", name=f"x{k}_{li}")
                    nc.sync.dma_start(
                        t[:], xT[k * 128 : (k + 1) * 128, start : start + width]
                    )
                    xt[(k, li)] = t
                    t = cpool.tile([128, width], f16, tag=f"h{k}_{li}", name=f"h{k}_{li}")
                    nc.sync.dma_start(
                        t[:], hT[k * 128 : (k + 1) * 128, start : start + width]
                    )
                    ht[(k, li)] = t

            load_inputs(0)
            load_inputs(1)

            for k in range(2):
                wB[k] = cpool.tile([128, 4 * D], f16, tag=f"wB{k}", name=f"wB{k}")
                nc.sync.dma_start(
                    wB[k][:], wcat[k * 128 : (k + 1) * 128, 2 * D : 6 * D]
                )
            b_sb = cpool.tile([128, 6], f32, tag="bcat")
            nc.sync.dma_start(b_sb[:], bcat[:, :])

            for li in range(2, len(PLAN)):
                load_inputs(li)

            def wap(i, k, g):
                """Weight AP [128,128] for matrix index i (order _WORDER),
                contraction half k, output-feature half g."""
                if i < 2:
                    return wA[k][:, i * D + g * 128 : i * D + (g + 1) * 128]
                return wB[k][:, (i - 2) * D + g * 128 : (i - 2) * D + (g + 1) * 128]

            og = {}
            for j in range(nsub):
                li, off = _sub_to_load(j)
                sl = slice(off, off + CH)
                xs = [xt[(k, li)][:, sl] for k in range(2)]
                hs = [ht[(k, li)][:, sl] for k in range(2)]

                def gate_psum(tag, wi, ui, rhs_u, g):
                    p = ppool.tile([128, CH], f32, tag=tag)
                    nc.tensor.matmul(p[:], wap(wi, 0, g), xs[0], start=True, stop=False)
                    nc.tensor.matmul(p[:], wap(wi, 1, g), xs[1], start=False, stop=False)
                    nc.tensor.matmul(p[:], wap(ui, 0, g), rhs_u[0], start=False, stop=False)
                    nc.tensor.matmul(p[:], wap(ui, 1, g), rhs_u[1], start=False, stop=True)
                    return p

                # reset gate -> r*h (needed before the candidate matmuls)
                rh = []
                for g in range(2):
                    pr = gate_psum(f"pr{g}", 0, 1, hs, g)
                    rt = wpool.tile([128, CH], f16, tag=f"r{g}")
                    nc.scalar.activation(rt[:], pr[:], AF.Sigmoid,
                                         bias=b_sb[:, g : g + 1])
                    t = wpool.tile([128, CH], f16, tag=f"rh{g}")
                    nc.vector.tensor_mul(t[:], rt[:], hs[g])
                    rh.append(t)

                # update gate
                zt = []
                for g in range(2):
                    pz = gate_psum(f"pz{g}", 2, 3, hs, g)
                    t = wpool.tile([128, CH], f16, tag=f"z{g}")
                    nc.scalar.activation(t[:], pz[:], AF.Sigmoid,
                                         bias=b_sb[:, 2 + g : 3 + g])
                    zt.append(t)

                # candidate + combine into the output staging tile
                jo = j % (OG // CH)
                if jo == 0:
                    for g in range(2):
                        og[g] = opool.tile([128, OG], f16, tag=f"o{g}", name=f"og{g}")
                for g in range(2):
                    ph = gate_psum(f"ph{g}", 4, 5, rh, g)
                    hh = wpool.tile([128, CH], f16, tag=f"hh{g}")
                    nc.scalar.activation(hh[:], ph[:], AF.Tanh,
                                         bias=b_sb[:, 4 + g : 5 + g])
                    d = wpool.tile([128, CH], f16, tag=f"d{g}")
                    nc.vector.tensor_sub(d[:], hs[g], hh[:])
                    m = wpool.tile([128, CH], f16, tag=f"m{g}")
                    nc.vector.tensor_mul(m[:], zt[g][:], d[:])
                    osl = og[g][:, jo * CH : (jo + 1) * CH]
                    nc.vector.tensor_add(osl, hh[:], m[:])
                if jo == OG // CH - 1:
                    c0 = (j + 1) * CH - OG
                    for g in range(2):
                        nc.sync.dma_start(
                            outT[g * 128 : (g + 1) * 128, c0 : c0 + OG], og[g][:]
                        )

    nc.compile()
    return nc


_NC_CACHE = {}


def _get_nc():
    key = (S, os.environ.get("GRU_MM_DTYPE", "float16"))
    if key not in _NC_CACHE:
        _NC_CACHE[key] = build_nc(S, key[1])
    return _NC_CACHE[key]


def _make_in_maps(inputs):
    f32 = np.float32
    dt16 = {"float16": np.float16}.get(
        os.environ.get("GRU_MM_DTYPE", "float16")
    )
    if dt16 is None:
        import ml_dtypes

        dt16 = ml_dtypes.bfloat16
    x = np.asarray(inputs["x"], f32)
    h = np.asarray(inputs["h_t_1"], f32)
    wcat = np.ascontiguousarray(
        np.concatenate(
            [np.asarray(inputs[n], f32) for n in ("Wr", "Ur", "Wz", "Uz", "Wh", "Uh")],
            axis=1,
        ).astype(dt16)
    )
    bcat = np.ascontiguousarray(
        np.concatenate(
            [np.asarray(inputs[n], f32).reshape(2, 128).T for n in ("br", "bz", "bh")],
            axis=1,
        )
    )
    consts = {"wcat": wcat, "bcat": bcat}
    in_maps = []
    for c in range(N_CORES):
        sl = slice(c * S, (c + 1) * S)
        m = {
            "xT": np.ascontiguousarray(x[sl].T.astype(dt16)),
            "hT": np.ascontiguousarray(h[sl].T.astype(dt16)),
        }
        m.update(consts)
        in_maps.append(m)
    return in_maps


def run(inputs, trace=False):
    """Run on hardware; returns (h_t ndarray, BassKernelResults)."""
    from concourse.bass_utils import run_bass_kernel_spmd

    nc = _get_nc()
    in_maps = _make_in_maps(inputs)
    res = run_bass_kernel_spmd(nc, in_maps, list(range(N_CORES)), trace=trace)
    out = np.empty((B, D), np.float32)
    for c in range(N_CORES):
        out[c * S : (c + 1) * S] = res.results[c]["outT"].T.astype(np.float32)
    return out, res


def kernel(**inputs):
    out, _ = run(inputs, trace=False)
    return (out, out)


# revision 4
# speedup vs baseline: 1.1829x; 1.0085x over previous
"""GRU cell on 8 Trainium2 NeuronCores.

Reference computation (B=65536, D=256):
    z = sigmoid(x@Wz + h@Uz + bz)
    r = sigmoid(x@Wr + h@Ur + br)
    h_hat = tanh(x@Wh + (r*h)@Uh + bh)
    h_t = z*h + (1-z)*h_hat  ; returns (h_t, h_t)

Strategy: data-parallel over the batch dim (8 shards of 8192 rows), all
fp16 on chip (rel_l2 ~1.1e-3 vs the f32 reference; gate is 2e-2):
  * host packs each shard as [128 partitions, 4 blocks, 8192] fp16 where
    the blocks are (x k0, x k1, h k0, h k1) - the contraction dim of all
    six GEMMs is the SBUF partition dim and one DMA fetches all four
    operand tiles of a column range (DMA triggers are ~645ns each on
    SyncE, so trigger count is latency that delays the pipeline head),
  * weights packed into one [256, 1536] fp16 matrix, r-gate slice first,
    DMA-ordered so the first matmul can start ~2.5us in,
  * all input tiles are SBUF-resident (8.4MB of 24MB), so DMA runs free
    of WAR hazards from t=0,
  * the r-gate of sub-chunk j+1 is computed one iteration early so its
    sigmoid+r*h (ScalarE+VectorE) never gate the candidate matmuls,
  * fp16 gate math on DVE (2x mode, SBUF-only), activations read PSUM
    f32 and write fp16, per-sub-chunk output DMAs keep the tail short.
"""

import os
import sys

for _p in ("/opt/trn_rl_repo", "/root/.axon_site/_ro/trn_rl_repo"):
    if os.path.isdir(_p) and _p not in sys.path:
        sys.path.append(_p)

import numpy as np

B = 65536
D = 256
N_CORES = 8
S = B // N_CORES  # batch rows per core
CH = 512  # batch columns per PSUM bank / compute sub-chunk

# Input-tile load plan: (col_start, width). The first is narrow (and split
# per block) so the pipeline head fills fast; the rest are wide packed
# loads for DMA efficiency.
PLAN = [(0, 512), (512, 512)] + [(1024 + 1024 * i, 1024) for i in range(7)]
# block order inside the packed input tensor
_BLOCKS = ("x0", "x1", "h0", "h1")
# matrix order inside the packed weight tensor
_WORDER = ("Wr", "Ur", "Wz", "Uz", "Wh", "Uh")
_BORDER = ("br", "bz", "bh")


def _sub_to_load(j):
    """Map 512-wide sub-chunk j to (load_index, local col offset)."""
    c0 = j * CH
    for li, (start, width) in enumerate(PLAN):
        if start <= c0 < start + width:
            return li, c0 - start
    raise ValueError(j)


def build_nc(s=S, mm_dtype_name=None):
    """Build + compile the per-core Bass program for a shard of s rows."""
    import concourse.bass as bass
    import concourse.mybir as mybir
    import concourse.tile as tile
    from concourse import bacc

    f32 = mybir.dt.float32
    if mm_dtype_name is None:
        mm_dtype_name = os.environ.get("GRU_MM_DTYPE", "float16")
    f16 = getattr(mybir.dt, mm_dtype_name)
    AF = mybir.ActivationFunctionType

    nc = bacc.Bacc("TRN2", target_bir_lowering=False)
    xh = nc.dram_tensor("xh", [128, 4, s], f16, kind="ExternalInput")
    wcat = nc.dram_tensor("wcat", [D, 6 * D], f16, kind="ExternalInput")
    bcat = nc.dram_tensor("bcat", [128, 6], f32, kind="ExternalInput")
    outT = nc.dram_tensor("outT", [D, s], f16, kind="ExternalOutput")

    nsub = s // CH

    with tile.TileContext(nc) as tc:
        with (
            tc.tile_pool(name="const", bufs=1) as cpool,
            tc.tile_pool(name="work", bufs=2) as wpool,
            tc.tile_pool(name="psum", bufs=1, space=bass.MemorySpace.PSUM) as ppool,
        ):
            inp = {}  # (block, load_idx) -> AP [128, width]

            # DMA issue order is latency-critical at the pipeline head:
            # r weights, x blocks of the first columns, z/h weights and h
            # blocks interleaved, then everything else.
            wA, wB = {}, {}
            for k in range(2):
                wA[k] = cpool.tile([128, 2 * D], f16, tag=f"wA{k}", name=f"wA{k}")
                nc.sync.dma_start(wA[k][:], wcat[k * 128 : (k + 1) * 128, 0 : 2 * D])

            w0, _ = PLAN[0]
            for bi, blk in enumerate(_BLOCKS):  # x0, x1, (wB0), h0, (wB1), h1
                t = cpool.tile([128, PLAN[0][1]], f16, tag=f"i{blk}_0",
                               name=f"i{blk}_0")
                nc.sync.dma_start(t[:], xh[:, bi, w0 : w0 + PLAN[0][1]])
                inp[(blk, 0)] = t
                if blk == "x1":
                    wB[0] = cpool.tile([128, 4 * D], f16, tag="wB0", name="wB0")
                    nc.sync.dma_start(
                        wB[0][:], wcat[0:128, 2 * D : 6 * D]
                    )
                if blk == "h0":
                    wB[1] = cpool.tile([128, 4 * D], f16, tag="wB1", name="wB1")
                    nc.sync.dma_start(
                        wB[1][:], wcat[128:256, 2 * D : 6 * D]
                    )
            b_sb = cpool.tile([128, 6], f32, tag="bcat")
            nc.sync.dma_start(b_sb[:], bcat[:, :])

            for li in range(1, len(PLAN)):
                start, width = PLAN[li]
                t = cpool.tile([128, 4, width], f16, tag=f"ixh_{li}",
                               name=f"ixh_{li}")
                nc.sync.dma_start(t[:], xh[:, :, start : start + width])
                for bi, blk in enumerate(_BLOCKS):
                    inp[(blk, li)] = t[:, bi, :]

            def wap(i, k, g):
                """Weight AP [128,128] for matrix index i (order _WORDER),
                contraction half k, output-feature half g."""
                if i < 2:
                    return wA[k][:, i * D + g * 128 : i * D + (g + 1) * 128]
                return wB[k][:, (i - 2) * D + g * 128 : (i - 2) * D + (g + 1) * 128]

            def operands(j):
                li, off = _sub_to_load(j)
                sl = slice(off, off + CH)
                xs = [inp[(f"x{k}", li)][:, sl] for k in range(2)]
                hs = [inp[(f"h{k}", li)][:, sl] for k in range(2)]
                return xs, hs

            def gate_psum(tag, wi, ui, xs, rhs_u, g):
                p = ppool.tile([128, CH], f32, tag=tag, name=tag)
                nc.tensor.matmul(p[:], wap(wi, 0, g), xs[0], start=True, stop=False)
                nc.tensor.matmul(p[:], wap(wi, 1, g), xs[1], start=False, stop=False)
                nc.tensor.matmul(p[:], wap(ui, 0, g), rhs_u[0], start=False, stop=False)
                nc.tensor.matmul(p[:], wap(ui, 1, g), rhs_u[1], start=False, stop=True)
                return p

            def r_gate(j):
                """reset gate -> r*h tiles for sub-chunk j."""
                xs, hs = operands(j)
                rh = []
                for g in range(2):
                    pr = gate_psum(f"pr{g}", 0, 1, xs, hs, g)
                    rt = wpool.tile([128, CH], f16, tag=f"r{g}", name=f"r{g}")
                    nc.scalar.activation(rt[:], pr[:], AF.Sigmoid,
                                         bias=b_sb[:, g : g + 1])
                    t = wpool.tile([128, CH], f16, tag=f"rh{g}", name=f"rh{g}")
                    nc.vector.tensor_mul(t[:], rt[:], hs[g])
                    rh.append(t)
                return rh

            # software pipeline: r-gate one sub-chunk ahead of z/candidate
            rh_cur = r_gate(0)
            for j in range(nsub):
                xs, hs = operands(j)
                rh_next = r_gate(j + 1) if j + 1 < nsub else None

                zt = []
                for g in range(2):
                    pz = gate_psum(f"pz{g}", 2, 3, xs, hs, g)
                    t = wpool.tile([128, CH], f16, tag=f"z{g}", name=f"z{g}")
                    nc.scalar.activation(t[:], pz[:], AF.Sigmoid,
                                         bias=b_sb[:, 2 + g : 3 + g])
                    zt.append(t)

                for g in range(2):
                    ph = gate_psum(f"ph{g}", 4, 5, xs, rh_cur, g)
                    hh = wpool.tile([128, CH], f16, tag=f"hh{g}", name=f"hh{g}")
                    nc.scalar.activation(hh[:], ph[:], AF.Tanh,
                                         bias=b_sb[:, 4 + g : 5 + g])
                    d = wpool.tile([128, CH], f16, tag=f"d{g}", name=f"d{g}")
                    nc.vector.tensor_sub(d[:], hs[g], hh[:])
                    m = wpool.tile([128, CH], f16, tag=f"m{g}", name=f"m{g}")
                    nc.vector.tensor_mul(m[:], zt[g][:], d[:])
                    o = wpool.tile([128, CH], f16, tag=f"o{g}", name=f"o{g}")
                    nc.vector.tensor_add(o[:], hh[:], m[:])
                    nc.sync.dma_start(
                        outT[g * 128 : (g + 1) * 128, j * CH : (j + 1) * CH], o[:]
                    )
                rh_cur = rh_next

    nc.compile()
    return nc


_NC_CACHE = {}


def _get_nc():
    key = (S, os.environ.get("GRU_MM_DTYPE", "float16"))
    if key not in _NC_CACHE:
        _NC_CACHE[key] = build_nc(S, key[1])
    return _NC_CACHE[key]


def _make_in_maps(inputs):
    f32 = np.float32
    dt16 = {"float16": np.float16}.get(
        os.environ.get("GRU_MM_DTYPE", "float16")
    )
    if dt16 is None:
        import ml_dtypes

        dt16 = ml_dtypes.bfloat16
    x = np.asarray(inputs["x"], f32)
    h = np.asarray(inputs["h_t_1"], f32)
    wcat = np.ascontiguousarray(
        np.concatenate(
            [np.asarray(inputs[n], f32) for n in ("Wr", "Ur", "Wz", "Uz", "Wh", "Uh")],
            axis=1,
        ).astype(dt16)
    )
    bcat = np.ascontiguousarray(
        np.concatenate(
            [np.asarray(inputs[n], f32).reshape(2, 128).T for n in ("br", "bz", "bh")],
            axis=1,
        )
    )
    consts = {"wcat": wcat, "bcat": bcat}
    in_maps = []
    for c in range(N_CORES):
        sl = slice(c * S, (c + 1) * S)
        xT = x[sl].T.astype(dt16)  # [256, S]
        hT = h[sl].T.astype(dt16)
        xh = np.empty((128, 4, S), dt16)
        xh[:, 0] = xT[0:128]
        xh[:, 1] = xT[128:256]
        xh[:, 2] = hT[0:128]
        xh[:, 3] = hT[128:256]
        m = {"xh": np.ascontiguousarray(xh)}
        m.update(consts)
        in_maps.append(m)
    return in_maps


def run(inputs, trace=False):
    """Run on hardware; returns (h_t ndarray, BassKernelResults)."""
    from concourse.bass_utils import run_bass_kernel_spmd

    nc = _get_nc()
    in_maps = _make_in_maps(inputs)
    res = run_bass_kernel_spmd(nc, in_maps, list(range(N_CORES)), trace=trace)
    out = np.empty((B, D), np.float32)
    for c in range(N_CORES):
        out[c * S : (c + 1) * S] = res.results[c]["outT"].T.astype(np.float32)
    return out, res


def kernel(**inputs):
    out, _ = run(inputs, trace=False)
    return (out, out)
